# revision 17
# baseline (speedup 1.0000x reference)
"""Trainium2 Bass kernel for nn_DensityVQC (batched 2-qubit VQC Z-expectation).

Algebra
-------
The reference builds rho_b = conj(psi_b) psi_b^T (note: transpose of the
standard density matrix), evolves rho' = U rho U^dag and returns
tr(rho' Z0) with Z0 = diag(1,1,-1,-1).  This collapses to a per-row
quadratic form: with V = conj(U) (the transposed-rho convention flips the
conjugation) and phi = V psi,

    out_b = |phi_0|^2 + |phi_1|^2 - |phi_2|^2 - |phi_3|^2
          = 2 * || C psi_b ||^2 - ||psi_b||^2        (C = V[0:2, :], U unitary)
          = || A r_b + B m_b ||^2 - 1                (inputs are unit-norm)

with real 4x4 matrices A = sqrt(2)*[Re C; Im C], B = sqrt(2)*[-Im C; Re C].
So the device kernel is: per batch row (r, m in R^4), compute w = W8^T x
(x = [r; m] in R^8, W8 = [A^T; B^T]), then out = sum(w^2) - 1.

Device mapping (per core, pure data parallel over 8 cores)
----------------------------------------------------------
fp16 end to end (tolerance is 2e-2; fp16 contributes ~1e-3):
host interleaves r/m into one [131072, 8] slab per core and lays it out
component-major as x [128 = 16 groups x 8 comps, 8192] fp16.  Consts ride
as the leading 96 columns (W block-diag [128, 64] + zsum [128, 32]).

Per pair p (supertiles 2p, 2p+1; 1024 input columns):
  1. PE: two matmuls with the SAME stationary kron(I16, W8) write
     phi[0:64] and phi[64:128] of one PSUM bank (PE column tiling,
     tile_position (0,0) / (0,64)).  The bank then holds the standard
     32-groups-x-4-comps layout.
  2. DVE (even p) / ACT (odd p): S = phi^2 -> SBUF fp16.
  3. PE: reduce matmul with stationary zsum = kron(I32, ones(4,1)) fp16,
     writing group sums into partitions [32*(p%4), +32) of a shared
     PSUM bank (PE column tiling again - no shifted-pattern consts).
  4. After 4 pairs: one full-width PSUM->SBUF copy folds the -1 and casts
     to fp16 (ACT for the first half, DVE for the second), then DMA out.
All input DMAs ride the sync queue (a single HWDGE queue stripes across
all 16 DMA engines, ~360 GB/s); outputs go out on the scalar queue.
Warm-up matmuls on a memset tile keep the PE p-state ramped through the
load window.  Host un-permutes the [2,128,512] fp16 output (pure data
marshalling) and applies the f32 cast.
"""

import sys
import numpy as np

if "/opt/trn_rl_repo" not in sys.path:
    sys.path.insert(0, "/opt/trn_rl_repo")

import concourse.bass as bass
import concourse.tile as tile
from concourse import bacc, mybir
from concourse import bass_utils
from concourse.tile_rust import add_dep_helper

N_CORES = 8
BSZ = 1_048_576
BC = BSZ // N_CORES            # 131072 rows per core
NCOL = BC // 16                # 8192 component-major free columns (16 rows/col)
N_PAIRS = 8                    # 8 pairs x 1024 columns
NCONST = 96                    # leading const columns: W block 64 + zsum 32
# The PE reaches full clock ~3us after its last idle gap ends (HAM
# p-state ramp), but full-clock grants are power-budgeted — warm cycles
# spend grant.  10 back-to-back warms bridge PE start (~7.7us) to the
# first consumable chunk (~9.9us) with minimal burn.
N_WARM = 12
# Keeper matmuls (256-col, on the memset tile) issued after each pair's
# reduce: they fill PE's DMA-wait gaps so the p-state ramp is never
# reset by starvation while the stream paces the pairs.
KEEPERS = [1, 1, 1, 1, 1, 1, 1, 0]
F16 = mybir.dt.float16
F32 = mybir.dt.float32
N_LAYERS = 6


def _circuit_unitary(ry, rz):
    """4x4 circuit unitary, float64 mirror of reference._circuit_unitary."""
    ry = np.asarray(ry, dtype=np.float64)
    rz = np.asarray(rz, dtype=np.float64)
    cnot = np.array(
        [[1, 0, 0, 0], [0, 1, 0, 0], [0, 0, 0, 1], [0, 0, 1, 0]],
        dtype=np.complex128,
    )

    def _ry(th):
        c, s = np.cos(th / 2), np.sin(th / 2)
        return np.array([[c, -s], [s, c]], dtype=np.complex128)

    def _rz(th):
        return np.diag([np.exp(-0.5j * th), np.exp(0.5j * th)])

    u = np.eye(4, dtype=np.complex128)
    for l in range(ry.shape[0]):
        ry_full = np.kron(_ry(ry[l, 0]), _ry(ry[l, 1]))
        rz_full = np.kron(_rz(rz[l, 0]), _rz(rz[l, 1]))
        u = cnot @ (rz_full @ (ry_full @ u))
    return u


def _host_consts(ry_params, rz_params):
    u = _circuit_unitary(ry_params, rz_params)
    c = np.conj(u)[0:2, :]
    a = np.sqrt(2.0) * np.vstack([c.real, c.imag])     # 4x4, w = A r + B m
    b = np.sqrt(2.0) * np.vstack([-c.imag, c.real])
    w8 = np.vstack([a.T, b.T])                         # [8,4]: w = W8^T x
    wblk = np.kron(np.eye(16), w8)                     # [128, 64]
    zsum = np.kron(np.eye(32), np.ones((4, 1)))        # [128, 32]
    ctile = np.concatenate([wblk, zsum], axis=1)       # [128, 96]
    return ctile.astype(np.float16)


def _to_component_major(x8):
    """x8 [BC, 8] -> [128, NCOL]: col j holds batch rows [16j, 16j+16);
    partition 8u+c = component c of row 16j+u."""
    return np.ascontiguousarray(x8.reshape(NCOL, 16, 8).transpose(1, 2, 0)
                                ).reshape(128, NCOL)


def _from_out(y):
    """y [2, 128, 512] -> [BC]: pair p = 4h+q wrote partitions
    [32q, 32q+32); within a stripe, g32 = 16*half + g addresses batch
    b = 16384p + 8192*half + 16n + g at column n."""
    return np.ascontiguousarray(
        y.reshape(2, 4, 2, 16, 512).transpose(0, 1, 2, 4, 3)
    ).reshape(-1)


# Input chunk boundaries (columns of the [128, NCONST + NCOL] tensor),
# aligned to 512-column pair halves (+NCONST) so pair p never waits on a
# chunk beyond its own columns.  Chunks alternate between the sync and
# scalar HWDGE queues so descriptor generation is not serialized on one
# engine; each queue's transfers stripe across all 16 DMA engines.  The
# consts and the first data half-pair ride separate queues in parallel
# so the first transform can start as early as possible.
CHUNKS = [0, 96, 608, 1632, 3168, 4704, 6240, 7776, NCONST + NCOL]


def _build_program():
    nc = bacc.Bacc("TRN2", target_bir_lowering=False, debug=False)
    xt_d = nc.dram_tensor("xt", [128, NCONST + NCOL], F16, kind="ExternalInput")
    out_d = nc.dram_tensor("out", [2, 128, 512], F16, kind="ExternalOutput")

    with tile.TileContext(nc) as tc:
        with (
            tc.tile_pool(name="io", bufs=1) as iopool,
            tc.tile_pool(name="work", bufs=3) as wpool,
            tc.tile_pool(name="psum", bufs=1, space=bass.MemorySpace.PSUM) as ppool,
        ):
            xt_t = iopool.tile([128, NCONST + NCOL], F16, name="xt_t")
            wblk = xt_t[:, 0:64]
            zsum = xt_t[:, 64:96]
            warm_t = iopool.tile([128, 256], F16, name="warm_t")
            out_lo = iopool.tile([128, 512], F16, name="out_lo")
            out_hi = iopool.tile([128, 512], F16, name="out_hi")

            # Warm-up: memset a tile on DVE, then spin the PE so its
            # p-state is ramped when the first data chunk lands.
            nc.vector.memset(warm_t[:], 0.0)
            pwarm = ppool.tile([128, 256], F32, name="pwarm", bufs=2)
            for w in range(N_WARM):
                nc.tensor.matmul(
                    pwarm[:], warm_t[:, 0:128], warm_t[:], start=True, stop=True
                )

            prev = {0: None, 1: None}
            for q in range(len(CHUNKS) - 1):
                cs = bass.ds(CHUNKS[q], CHUNKS[q + 1] - CHUNKS[q])
                eng = nc.sync if q % 2 == 0 else nc.scalar
                dma = eng.dma_start(xt_t[:, cs], xt_d.ap()[:, cs])
                # Ordering-only edges keep each FIFO queue in chunk order.
                if prev[q % 2] is not None:
                    add_dep_helper(dma.ins, prev[q % 2].ins, sync=False, reason="q")
                prev[q % 2] = dma

            obank_a = ppool.tile([128, 512], F32, name="obank_a", bufs=1)
            obank_b = ppool.tile([128, 512], F32, name="obank_b", bufs=1)

            for p in range(N_PAIRS):
                c0 = NCONST + 1024 * p
                phi = ppool.tile([128, 512], F32, name="phi", bufs=3)
                nc.tensor.matmul(
                    phi[0:64, :], wblk, xt_t[:, c0 : c0 + 512],
                    start=True, stop=True,
                )
                nc.tensor.matmul(
                    phi[64:128, :], wblk, xt_t[:, c0 + 512 : c0 + 1024],
                    start=True, stop=True,
                )

                s_sb = wpool.tile([128, 512], F16, name="s_sb")
                nc.scalar.activation(
                    s_sb[:], phi[:], mybir.ActivationFunctionType.Square
                )

                obank = obank_a if p < 4 else obank_b
                q = p % 4
                nc.tensor.matmul(
                    obank[32 * q : 32 * (q + 1), :], zsum, s_sb[:],
                    start=True, stop=True, tile_position=(0, 32 * q),
                )

                if p == 3:
                    nc.vector.tensor_scalar_add(out_lo[:], obank_a[:], -1.0)
                    d_lo = nc.scalar.dma_start(out_d.ap()[0], out_lo[:])
                    add_dep_helper(d_lo.ins, prev[1].ins, sync=False, reason="q")
                if p == 7:
                    # ACT is idle by now and its copy is faster than DVE's.
                    nc.scalar.activation(
                        out_hi[:], obank_b[:],
                        mybir.ActivationFunctionType.Copy, bias=-1.0,
                    )
                    d_hi = nc.sync.dma_start(out_d.ap()[1], out_hi[:])
                    add_dep_helper(d_hi.ins, prev[0].ins, sync=False, reason="q")

                for _ in range(KEEPERS[p]):
                    nc.tensor.matmul(
                        pwarm[:], warm_t[:, 0:128], warm_t[:],
                        start=True, stop=True,
                    )
    nc.compile()
    return nc


_PROG_CACHE = None


def _get_program():
    global _PROG_CACHE
    if _PROG_CACHE is None:
        _PROG_CACHE = _build_program()
    return _PROG_CACHE


def _run(ry_params, rz_params, states_real, states_imag, **hw_kwargs):
    ctile = _host_consts(ry_params, rz_params)
    x8 = np.empty((BSZ, 8), dtype=np.float16)
    x8[:, 0:4] = states_real
    x8[:, 4:8] = states_imag
    in_maps = []
    for k in range(N_CORES):
        xc = _to_component_major(x8[k * BC : (k + 1) * BC])
        in_maps.append({"xt": np.concatenate([ctile, xc], axis=1)})
    nc = _get_program()
    res = bass_utils.run_bass_kernel_spmd(
        nc, in_maps, core_ids=list(range(N_CORES)), **hw_kwargs
    )
    out = np.concatenate(
        [_from_out(res.results[k]["out"]) for k in range(N_CORES)]
    ).astype(np.float32)
    return out, res


def kernel(ry_params, rz_params, states_real, states_imag):
    out, _ = _run(ry_params, rz_params, states_real, states_imag)
    return out
